# revision 5
# baseline (speedup 1.0000x reference)
"""Differentiable envelope follower on 8 Trainium2 NeuronCores.

Algorithm: the per-sample recurrence
    env[t] = c[t]*env[t-1] + (1-c[t])*|x[t]|,  c[t] = ca if |x[t]| > env[t-1] else cr
is solved by policy iteration: derive attack/release decisions elementwise
from a trajectory guess, solve the resulting LINEAR recurrence exactly with
tensor_tensor_scan (a DVE-only instruction), repeat.  A causal EMA proxy
(gamma-scaled running mean of |x|) provides the initial guess, which makes
TWO solves sufficient (max rel err ~3e-4 vs the 2e-2 gate, validated in fp16
at full scale and on an independent random draw).

Per solve, each elementwise pass is a single legal instruction:
    c  = cr + dc*dec      (Act activation, Copy with scale/bias)
    kk = 1-c = kr+|dc|dec (tensor_scalar, two scalars)
    d1 = kk*lt            (tensor_tensor mult; lt = |x| f16 plane)
Solve-0 decisions are an all-f16 2x-mode is_gt on DVE against the proxy.
Solve-1 decisions are stored as sign(|x[t]|-env[t-1]) in {-1,0,+1}: Pool does
the subtract, Act the Sign (comparison ops are DVE-only, so this keeps the
second decision pass off the DVE whose three scans are the critical budget).
Sign ties (diff==0) are exactly neutral since c+kk=1.

Engine budget per core (cost model): DVE = EMA scan + solve scans + one 2x
compare (~115us); Act = abs + 2 c-passes + Sign (~118us); Pool = products +
diffs + kk (~127us).  y is stored f16 (upcast on host) to halve store traffic
on the shared DMA bus; x loads stream on the SP queue.

Sharding: L=480000 split across 8 cores (60000 each); 64 batch rows x 2
L-halves fill the 128 SBUF partitions ([128, 30000] per core).  Chunk-boundary
states are exchanged between solves with a tiny AllGather (512B).
"""

import math
import numpy as np

# ---- problem constants (hardcoded per contract) ----
B = 64
L = 480000
NCORES = 8
KCORE = L // NCORES          # 60000 per core
HALF = KCORE // 2            # 30000 per partition-row
P = 128

# ---- tunables ----
YS = 1024.0                  # y stored as YS*env in f16 (avoids subnormals)
TF = 1200                    # free-dim tile size (must divide HALF)
ITERS = 2                    # number of linear solves (policy iterations)
EQ = 1.42                    # equilibrium level of the init-guess ramp
TAU = 5600.0                 # time constant of the init-guess ramp
CM_SAMPLES = 3000.0          # EMA proxy time constant (samples)
GAMMA = 1.8                  # proxy -> envelope threshold scale
PD_POOL = 4                  # pd on Pool every j%PD_POOL==3 (else DVE)
KK0_ACT = 3                  # kk0 on Act every j%KK0_ACT==0 (else Pool)

_RUN_KWARGS = {}             # test.py can set {"trace": True}
_cache = {}


def _coeffs(raw_attack, raw_release, sample_rate):
    # Mirror reference._coefficients exactly (same jax ops, on CPU).
    import jax
    import jax.numpy as jnp

    with jax.default_device(jax.devices("cpu")[0]):
        attack_ms = 0.1 + jax.nn.sigmoid(jnp.asarray(np.float32(raw_attack))) * 49.9
        release_ms = 10.0 + jax.nn.sigmoid(jnp.asarray(np.float32(raw_release))) * 490.0
        attack_samples = attack_ms * float(sample_rate) / 1000.0
        release_samples = release_ms * float(sample_rate) / 1000.0
        ca = jnp.exp(-1.0 / attack_samples)
        cr = jnp.exp(-1.0 / release_samples)
        return float(ca), float(cr)


def _build(ca, cr):
    import concourse.bass as bass
    import concourse.bacc as bacc
    import concourse.tile as tile
    from concourse import mybir
    from concourse.tile_rust import add_dep_helper
    from contextlib import ExitStack

    f32 = mybir.dt.float32
    f16 = mybir.dt.float16
    Alu = mybir.AluOpType
    Act = mybir.ActivationFunctionType
    NT = HALF // TF
    assert NT * TF == HALF

    dc = float(np.float32(ca) - np.float32(cr))          # < 0
    adc = -dc                                            # |dc| = cr - ca
    kr = float(np.float32(1.0) - np.float32(cr))
    cm = float(np.float32(math.exp(-1.0 / CM_SAMPLES)))
    YSf = np.float32(YS)
    gcm = float(np.float32(GAMMA) * (np.float32(1.0) - np.float32(cm)))
    # sign-convention (s in {-1,0,1}) affine constants for solve 1
    c1_bias = float(np.float32(cr) + np.float32(dc) / 2)
    c1_scale = float(np.float32(dc) / 2)
    k1_bias = float(YS * (np.float32(kr) + np.float32(adc) / 2))
    k1_scale = float(YS * (np.float32(adc) / 2))

    nc = bacc.Bacc("TRN2", target_bir_lowering=False, debug=False,
                   num_devices=NCORES)

    x_in = nc.dram_tensor("xc", [P, HALF], f32, kind="ExternalInput")
    seed_in = nc.dram_tensor("seed0", [P, 1], f32, kind="ExternalInput")
    selw_in = nc.dram_tensor("selw", [P, NCORES], f32, kind="ExternalInput")
    y_out = nc.dram_tensor("yc", [P, HALF], f16, kind="ExternalOutput")
    bnd_loc = nc.dram_tensor("bnd_loc", [P], f32)
    bnd_all = nc.dram_tensor("bnd_all", [NCORES, P], f32, addr_space="Shared")

    nc.alloc_semaphore("bnd_dma")
    nc.alloc_semaphore("bnd_cc")
    groups = [list(range(NCORES))]

    with tile.TileContext(nc) as tc:
        with ExitStack() as ctx:
            pers = ctx.enter_context(tc.tile_pool(name="pers", bufs=1))
            xp = ctx.enter_context(tc.tile_pool(name="x", bufs=2))
            pdp = ctx.enter_context(tc.tile_pool(name="pd", bufs=2))
            pgp = ctx.enter_context(tc.tile_pool(name="pg", bufs=2))
            dbp = ctx.enter_context(tc.tile_pool(name="db", bufs=2))
            cp = ctx.enter_context(tc.tile_pool(name="c", bufs=2))
            kkp = ctx.enter_context(tc.tile_pool(name="kk", bufs=2))
            d1p = ctx.enter_context(tc.tile_pool(name="d1", bufs=2))
            efp = ctx.enter_context(tc.tile_pool(name="ef", bufs=2))
            dfp = ctx.enter_context(tc.tile_pool(name="df", bufs=2))
            yhp = ctx.enter_context(tc.tile_pool(name="yh", bufs=3))
            bcolp = ctx.enter_context(tc.tile_pool(name="bcol", bufs=2))
            seedp = ctx.enter_context(tc.tile_pool(name="seed", bufs=2))

            ltp = pers.tile([P, HALF], f16, tag="lt")     # |x|
            decp = pers.tile([P, HALF], f16, tag="dec")   # sign(|x|-env) plane
            cmt = pers.tile([P, TF], f32, tag="cmt")      # EMA scan multiplier
            selw_sb = pers.tile([P, NCORES], f32, tag="selw")
            bnd_sb = pers.tile([P, NCORES], f32, tag="bnd")
            sel_t = pers.tile([P, NCORES], f32, tag="sel")

            seed0_t = seedp.tile([P, 1], f32, tag="s0")
            nc.gpsimd.dma_start(seed0_t[:, :], seed_in[:, :])
            nc.gpsimd.dma_start(selw_sb[:, :], selw_in[:, :])
            nc.gpsimd.memset(cmt[:, :], cm)

            # ---------- solve 0 + proxy ----------
            prev_pg = None
            prev_env = None
            env_last = None
            for j in range(NT):
                a = j * TF
                lts = ltp[:, a:a + TF]
                x_t = xp.tile([P, TF], f32, tag="x")
                nc.sync.dma_start(x_t[:, :], x_in[:, a:a + TF])
                nc.scalar.activation(lts, x_t[:, :], Act.Abs)

                pd_t = pdp.tile([P, TF], f16, tag="pd")
                pd_eng = nc.gpsimd if j % PD_POOL == PD_POOL - 1 else nc.vector
                pd_eng.tensor_scalar(pd_t[:, :], lts, gcm, None, op0=Alu.mult)
                pg_t = pgp.tile([P, TF], f16, tag="pg")
                pg_init = (prev_pg[:, TF - 1:TF] if j > 0 else seed0_t[:, 0:1])
                nc.vector.tensor_tensor_scan(pg_t[:, :], cmt[:, :], pd_t[:, :],
                                             pg_init, op0=Alu.mult, op1=Alu.add)

                # solve-0 decisions: lt > gamma-EMA (shifted), all-f16 DVE 2x
                db_t = dbp.tile([P, TF], f16, tag="db")
                nc.vector.tensor_tensor(db_t[:, 1:], lts[:, 1:],
                                        pg_t[:, :TF - 1], op=Alu.is_gt)
                prev_col = (prev_pg[:, TF - 1:TF] if j > 0 else seed0_t[:, 0:1])
                nc.vector.tensor_tensor(db_t[:, 0:1], lts[:, 0:1], prev_col,
                                        op=Alu.is_gt)
                prev_pg = pg_t

                c_t = cp.tile([P, TF], f32, tag="c")
                nc.scalar.activation(c_t[:, :], db_t[:, :], Act.Copy,
                                     bias=float(cr), scale=dc)
                kk_t = kkp.tile([P, TF], f32, tag="kk")
                if j % KK0_ACT == 0:
                    nc.scalar.activation(kk_t[:, :], db_t[:, :], Act.Copy,
                                         bias=float(kr), scale=adc)
                else:
                    nc.gpsimd.tensor_scalar(kk_t[:, :], db_t[:, :], adc,
                                            float(kr), op0=Alu.mult,
                                            op1=Alu.add)
                d1_t = d1p.tile([P, TF], f32, tag="d1")
                nc.gpsimd.tensor_tensor(d1_t[:, :], kk_t[:, :], lts,
                                        op=Alu.mult)

                env_t = efp.tile([P, TF], f32, tag="ef")
                init_ap = (prev_env[:, TF - 1:TF] if j > 0 else seed0_t[:, 0:1])
                nc.vector.tensor_tensor_scan(env_t[:, :], c_t[:, :],
                                             d1_t[:, :], init_ap,
                                             op0=Alu.mult, op1=Alu.add)

                # solve-1 decision inputs: diff = lt - env (shifted), then Sign
                df_t = dfp.tile([P, TF], f32, tag="df")
                nc.gpsimd.tensor_tensor(df_t[:, 1:], lts[:, 1:],
                                        env_t[:, :TF - 1], op=Alu.subtract)
                prev_ecol = (prev_env[:, TF - 1:TF] if j > 0
                             else seed0_t[:, 0:1])
                nc.gpsimd.tensor_tensor(df_t[:, 0:1], lts[:, 0:1], prev_ecol,
                                        op=Alu.subtract)
                nc.scalar.activation(decp[:, a:a + TF], df_t[:, :], Act.Sign)
                prev_env = env_t
                env_last = env_t

            # ---------- boundary exchange ----------
            bcol = bcolp.tile([P, 1], f32, tag="bcol")
            nc.vector.tensor_copy(bcol[:, :], env_last[:, TF - 1:TF])
            st1 = nc.gpsimd.dma_start(bnd_loc[0:64], bcol[64:128, 0:1])
            st2 = nc.gpsimd.dma_start(bnd_loc[64:128], bcol[0:64, 0:1])
            cc = nc.gpsimd.collective_compute(
                "AllGather", mybir.AluOpType.bypass,
                replica_groups=groups,
                ins=[bnd_loc[:]], outs=[bnd_all[:, :]],
            )
            add_dep_helper(cc.ins, st1.ins, sync=True,
                           reason="collective after bnd stores")
            add_dep_helper(cc.ins, st2.ins, sync=True,
                           reason="collective after bnd stores")
            for g in range(NCORES):
                ld = nc.gpsimd.dma_start(bnd_sb[:, g:g + 1], bnd_all[g, :])
                add_dep_helper(ld.ins, cc.ins, sync=True,
                               reason="bnd load after collective")
            nc.vector.tensor_tensor(sel_t[:, :], bnd_sb[:, :], selw_sb[:, :],
                                    op=Alu.mult)
            seed_t = seedp.tile([P, 1], f32, tag="sx")
            nc.vector.tensor_reduce(seed_t[:, :], sel_t[:, :],
                                    axis=mybir.AxisListType.X, op=Alu.add)
            # refresh tile-0 col-0 decision with the fresh seed
            dcol = bcolp.tile([P, 1], f32, tag="dcol")
            nc.gpsimd.tensor_tensor(dcol[:, :], ltp[:, 0:1], seed_t[:, 0:1],
                                    op=Alu.subtract)
            nc.scalar.activation(decp[:, 0:1], dcol[:, :], Act.Sign)

            # ---------- solve 1 (final) ----------
            seedy_t = seedp.tile([P, 1], f32, tag="sy")
            nc.vector.tensor_scalar(seedy_t[:, :], seed_t[:, :], float(YSf),
                                    None, op0=Alu.mult)
            prev_y = None
            for j in range(NT):
                a = j * TF
                lts = ltp[:, a:a + TF]
                dsl = decp[:, a:a + TF]
                c_t = cp.tile([P, TF], f32, tag="c")
                nc.scalar.activation(c_t[:, :], dsl, Act.Copy,
                                     bias=c1_bias, scale=c1_scale)
                kk_t = kkp.tile([P, TF], f32, tag="kk")
                nc.gpsimd.tensor_scalar(kk_t[:, :], dsl, k1_scale, k1_bias,
                                        op0=Alu.mult, op1=Alu.add)
                d1_t = d1p.tile([P, TF], f32, tag="d1")
                nc.gpsimd.tensor_tensor(d1_t[:, :], kk_t[:, :], lts,
                                        op=Alu.mult)
                y_t = yhp.tile([P, TF], f16, tag="yh")
                init_ap = (prev_y[:, TF - 1:TF] if j > 0 else seedy_t[:, 0:1])
                nc.vector.tensor_tensor_scan(y_t[:, :], c_t[:, :], d1_t[:, :],
                                             init_ap, op0=Alu.mult,
                                             op1=Alu.add)
                nc.gpsimd.dma_start(y_out[:, a:a + TF], y_t[:, :])
                prev_y = y_t
    nc.finalize()
    return nc


def _in_maps(x, ca, cr):
    x = np.ascontiguousarray(np.asarray(x, dtype=np.float32))
    maps = []
    t0 = np.empty(P, np.float64)
    for c in range(NCORES):
        t0[:64] = c * KCORE
        t0[64:] = c * KCORE + HALF
        seed0 = (EQ * (1.0 - np.exp(-t0 / TAU))).astype(np.float32)[:, None]
        selw = np.zeros((P, NCORES), np.float32)
        if c > 0:
            selw[:64, c - 1] = 1.0
        selw[64:, c] = 1.0
        s = c * KCORE
        xc = np.concatenate([x[:, s:s + HALF], x[:, s + HALF:s + KCORE]],
                            axis=0)
        maps.append({
            "xc": np.ascontiguousarray(xc),
            "seed0": seed0,
            "selw": selw,
        })
    return maps


def kernel(x, raw_attack, raw_release, sample_rate):
    from concourse.bass_utils import run_bass_kernel_spmd

    ca, cr = _coeffs(raw_attack, raw_release, sample_rate)
    key = (round(ca, 12), round(cr, 12), TF, ITERS, GAMMA, CM_SAMPLES)
    if key not in _cache:
        _cache[key] = _build(ca, cr)
    nc = _cache[key]

    maps = _in_maps(x, ca, cr)
    res = run_bass_kernel_spmd(nc, maps, list(range(NCORES)), **_RUN_KWARGS)
    kernel.last_results = res

    y = np.empty((B, L), np.float32)
    for c in range(NCORES):
        yc = np.asarray(res.results[c]["yc"], dtype=np.float32) * np.float32(1.0 / YS)
        s = c * KCORE
        y[:, s:s + HALF] = yc[:64]
        y[:, s + HALF:s + KCORE] = yc[64:]
    return y


# revision 8
# speedup vs baseline: 1.1462x; 1.1462x over previous
"""Differentiable envelope follower on 8 Trainium2 NeuronCores.

Algorithm: the per-sample recurrence
    env[t] = c[t]*env[t-1] + (1-c[t])*|x[t]|,  c[t] = ca if |x[t]| > env[t-1] else cr
is solved by policy iteration: derive attack/release decisions elementwise
from a trajectory guess, solve the resulting LINEAR recurrence exactly with
tensor_tensor_scan (a DVE-only instruction), repeat.  A causal EMA proxy
(gamma-scaled running mean of |x|) provides the initial guess, which makes
TWO solves sufficient (max rel err ~3e-4 vs the 2e-2 gate, validated in fp16
at full scale and on an independent random draw).

Per solve, each elementwise pass is a single legal instruction:
    c  = cr + dc*dec      (Act activation, Copy with scale/bias)
    kk = 1-c = kr+|dc|dec (tensor_scalar, two scalars)
    d1 = kk*lt            (tensor_tensor mult; lt = |x| f16 plane)
Solve-0 decisions are an all-f16 2x-mode is_gt on DVE against the proxy.
Solve-1 decisions are stored as sign(|x[t]|-env[t-1]) in {-1,0,+1}: Pool does
the subtract, Act the Sign (comparison ops are DVE-only, so this keeps the
second decision pass off the DVE whose three scans are the critical budget).
Sign ties (diff==0) are exactly neutral since c+kk=1.

Engine budget per core (cost model): DVE = EMA scan + solve scans + one 2x
compare (~115us); Act = abs + 2 c-passes + Sign (~118us); Pool = products +
diffs + kk (~127us).  y is stored f16 (upcast on host) to halve store traffic
on the shared DMA bus; x loads stream on the SP queue.

Sharding: L=480000 split across 8 cores (60000 each); 64 batch rows x 2
L-halves fill the 128 SBUF partitions ([128, 30000] per core).  Chunk-boundary
states are exchanged between solves with a tiny AllGather (512B).
"""

import math
import numpy as np

# ---- problem constants (hardcoded per contract) ----
B = 64
L = 480000
NCORES = 8
KCORE = L // NCORES          # 60000 per core
HALF = KCORE // 2            # 30000 per partition-row
P = 128

# ---- tunables ----
YS = 1024.0                  # y stored as YS*env in f16 (avoids subnormals)
TF = 1000                    # free-dim tile size (must divide HALF)
ITERS = 2                    # number of linear solves (policy iterations)
EQ = 1.42                    # equilibrium level of the init-guess ramp
TAU = 5600.0                 # time constant of the init-guess ramp
CM_SAMPLES = 3000.0          # EMA proxy time constant (samples)
GAMMA = 1.8                  # proxy -> envelope threshold scale
DEC_DVE = 3                  # solve-1 dec on DVE is_gt every j%DEC_DVE==0 (else Pool diff + Act Sign)

_RUN_KWARGS = {}             # test.py can set {"trace": True}
_cache = {}


def _coeffs(raw_attack, raw_release, sample_rate):
    # Mirror reference._coefficients exactly (same jax ops, on CPU).
    import jax
    import jax.numpy as jnp

    with jax.default_device(jax.devices("cpu")[0]):
        attack_ms = 0.1 + jax.nn.sigmoid(jnp.asarray(np.float32(raw_attack))) * 49.9
        release_ms = 10.0 + jax.nn.sigmoid(jnp.asarray(np.float32(raw_release))) * 490.0
        attack_samples = attack_ms * float(sample_rate) / 1000.0
        release_samples = release_ms * float(sample_rate) / 1000.0
        ca = jnp.exp(-1.0 / attack_samples)
        cr = jnp.exp(-1.0 / release_samples)
        return float(ca), float(cr)


def _build(ca, cr):
    import concourse.bass as bass
    import concourse.bacc as bacc
    import concourse.tile as tile
    from concourse import mybir
    from concourse.tile_rust import add_dep_helper
    from contextlib import ExitStack

    f32 = mybir.dt.float32
    f16 = mybir.dt.float16
    Alu = mybir.AluOpType
    Act = mybir.ActivationFunctionType
    NT = HALF // TF
    assert NT * TF == HALF

    dc = float(np.float32(ca) - np.float32(cr))          # < 0
    adc = -dc                                            # |dc| = cr - ca
    kr = float(np.float32(1.0) - np.float32(cr))
    cm = float(np.float32(math.exp(-1.0 / CM_SAMPLES)))
    YSf = np.float32(YS)
    gcm = float(np.float32(GAMMA) * (np.float32(1.0) - np.float32(cm)))
    # sign-convention (s in {-1,0,1}) affine constants for solve 1
    c1_bias = float(np.float32(cr) + np.float32(dc) / 2)
    c1_scale = float(np.float32(dc) / 2)
    k1_bias = float(YS * (np.float32(kr) + np.float32(adc) / 2))
    k1_scale = float(YS * (np.float32(adc) / 2))

    nc = bacc.Bacc("TRN2", target_bir_lowering=False, debug=False,
                   num_devices=NCORES)

    x_in = nc.dram_tensor("xc", [P, HALF], f32, kind="ExternalInput")
    seed_in = nc.dram_tensor("seed0", [P, 1], f32, kind="ExternalInput")
    selw_in = nc.dram_tensor("selw", [P, NCORES], f32, kind="ExternalInput")
    y_out = nc.dram_tensor("yc", [P, HALF], f16, kind="ExternalOutput")
    bnd_loc = nc.dram_tensor("bnd_loc", [P], f32)
    bnd_all = nc.dram_tensor("bnd_all", [NCORES, P], f32, addr_space="Shared")

    nc.alloc_semaphore("bnd_dma")
    nc.alloc_semaphore("bnd_cc")
    groups = [list(range(NCORES))]

    with tile.TileContext(nc) as tc:
        with ExitStack() as ctx:
            pers = ctx.enter_context(tc.tile_pool(name="pers", bufs=1))
            xp = ctx.enter_context(tc.tile_pool(name="x", bufs=2))
            pdp = ctx.enter_context(tc.tile_pool(name="pd", bufs=2))
            pgp = ctx.enter_context(tc.tile_pool(name="pg", bufs=2))
            dbp = ctx.enter_context(tc.tile_pool(name="db", bufs=2))
            cp = ctx.enter_context(tc.tile_pool(name="c", bufs=3))
            kkp = ctx.enter_context(tc.tile_pool(name="kk", bufs=3))
            d1p = ctx.enter_context(tc.tile_pool(name="d1", bufs=3))
            efp = ctx.enter_context(tc.tile_pool(name="ef", bufs=3))
            dfp = ctx.enter_context(tc.tile_pool(name="df", bufs=2))
            yhp = ctx.enter_context(tc.tile_pool(name="yh", bufs=2))
            bcolp = ctx.enter_context(tc.tile_pool(name="bcol", bufs=2))
            seedp = ctx.enter_context(tc.tile_pool(name="seed", bufs=2))

            ltp = pers.tile([P, HALF], f16, tag="lt")     # |x|
            decp = pers.tile([P, HALF], f16, tag="dec")   # sign(|x|-env) plane
            cmt = pers.tile([P, TF], f32, tag="cmt")      # EMA scan multiplier
            selw_sb = pers.tile([P, NCORES], f32, tag="selw")
            bnd_sb = pers.tile([P, NCORES], f32, tag="bnd")
            sel_t = pers.tile([P, NCORES], f32, tag="sel")

            seed0_t = seedp.tile([P, 1], f32, tag="s0")
            nc.gpsimd.dma_start(seed0_t[:, :], seed_in[:, :])
            nc.gpsimd.dma_start(selw_sb[:, :], selw_in[:, :])
            nc.gpsimd.memset(cmt[:, :], cm)

            # ---------- solve 0 + proxy ----------
            # emission is software-pipelined one step: tile j's scan/decision
            # ops are emitted while tile j+1's prep runs, so no engine queue
            # blocks at an unready instruction.
            prep = {}

            def emit_prep(j):
                a = j * TF
                lts = ltp[:, a:a + TF]
                x_t = xp.tile([P, TF], f32, tag="x")
                nc.sync.dma_start(x_t[:, :], x_in[:, a:a + TF])
                nc.scalar.activation(lts, x_t[:, :], Act.Abs)
                pd_t = pdp.tile([P, TF], f16, tag="pd")
                nc.gpsimd.tensor_scalar(pd_t[:, :], lts, gcm, None,
                                        op0=Alu.mult)
                pg_t = pgp.tile([P, TF], f16, tag="pg")
                pg_init = (prep[j - 1]["pg"][:, TF - 1:TF] if j > 0
                           else seed0_t[:, 0:1])
                nc.vector.tensor_tensor_scan(pg_t[:, :], cmt[:, :],
                                             pd_t[:, :], pg_init,
                                             op0=Alu.mult, op1=Alu.add)
                db_t = dbp.tile([P, TF], f16, tag="db")
                nc.vector.tensor_tensor(db_t[:, 1:], lts[:, 1:],
                                        pg_t[:, :TF - 1], op=Alu.is_gt)
                prev_col = (prep[j - 1]["pg"][:, TF - 1:TF] if j > 0
                            else seed0_t[:, 0:1])
                nc.vector.tensor_tensor(db_t[:, 0:1], lts[:, 0:1], prev_col,
                                        op=Alu.is_gt)
                c_t = cp.tile([P, TF], f32, tag="c")
                nc.scalar.activation(c_t[:, :], db_t[:, :], Act.Copy,
                                     bias=float(cr), scale=dc)
                kk_t = kkp.tile([P, TF], f32, tag="kk")
                nc.gpsimd.tensor_scalar(kk_t[:, :], db_t[:, :], adc,
                                        float(kr), op0=Alu.mult, op1=Alu.add)
                d1_t = d1p.tile([P, TF], f32, tag="d1")
                nc.gpsimd.tensor_tensor(d1_t[:, :], kk_t[:, :], lts,
                                        op=Alu.mult)
                prep[j] = {"pg": pg_t, "c": c_t, "d1": d1_t}
                prep.pop(j - 2, None)

            envs = {}

            def emit_scan(j):
                a = j * TF
                lts = ltp[:, a:a + TF]
                env_t = efp.tile([P, TF], f32, tag="ef")
                init_ap = (envs[j - 1][:, TF - 1:TF] if j > 0
                           else seed0_t[:, 0:1])
                nc.vector.tensor_tensor_scan(env_t[:, :], prep[j]["c"][:, :],
                                             prep[j]["d1"][:, :], init_ap,
                                             op0=Alu.mult, op1=Alu.add)
                # solve-1 decisions (shifted): DVE is_gt on some tiles
                # ({0,1} convention), Pool diff + Act Sign on the rest
                # ({-1,0,1} convention); solve-1 constants match per tile.
                if j % DEC_DVE == 0:
                    nc.vector.tensor_tensor(decp[:, a + 1:a + TF], lts[:, 1:],
                                            env_t[:, :TF - 1], op=Alu.is_gt)
                    pcol = (envs[j - 1][:, TF - 1:TF] if j > 0
                            else seed0_t[:, 0:1])
                    nc.vector.tensor_tensor(decp[:, a:a + 1], lts[:, 0:1],
                                            pcol, op=Alu.is_gt)
                else:
                    df_t = dfp.tile([P, TF], f32, tag="df")
                    nc.gpsimd.tensor_tensor(df_t[:, 1:], lts[:, 1:],
                                            env_t[:, :TF - 1],
                                            op=Alu.subtract)
                    nc.gpsimd.tensor_tensor(df_t[:, 0:1], lts[:, 0:1],
                                            envs[j - 1][:, TF - 1:TF],
                                            op=Alu.subtract)
                    nc.scalar.activation(decp[:, a:a + TF], df_t[:, :],
                                         Act.Sign)
                envs[j] = env_t
                envs.pop(j - 2, None)

            emit_prep(0)
            for j in range(1, NT):
                emit_prep(j)
                emit_scan(j - 1)
            emit_scan(NT - 1)
            env_last = envs[NT - 1]

            # ---------- solve-1 prep (emitted before the exchange so all
            # engines stay busy during the 15us collective) ----------
            prep1 = {}

            def emit_prep1(j):
                a = j * TF
                lts = ltp[:, a:a + TF]
                dsl = decp[:, a:a + TF]
                if j % DEC_DVE == 0:    # {0,1} decisions
                    cb, cs = float(cr), dc
                    kb, ks = float(YS * np.float32(kr)), float(YS * np.float32(adc))
                else:                    # sign {-1,0,1} decisions
                    cb, cs = c1_bias, c1_scale
                    kb, ks = k1_bias, k1_scale
                c_t = cp.tile([P, TF], f32, tag="c")
                nc.scalar.activation(c_t[:, :], dsl, Act.Copy, bias=cb,
                                     scale=cs)
                kk_t = kkp.tile([P, TF], f32, tag="kk")
                nc.vector.tensor_scalar(kk_t[:, :], dsl, ks, kb,
                                        op0=Alu.mult, op1=Alu.add)
                d1_t = d1p.tile([P, TF], f32, tag="d1")
                d1_eng = nc.vector if 2 <= j <= 5 else nc.gpsimd
                d1_eng.tensor_tensor(d1_t[:, :], kk_t[:, :], lts,
                                     op=Alu.mult)
                prep1[j] = {"c": c_t, "d1": d1_t}

            LOOK = 2
            for j in range(min(LOOK, NT)):
                emit_prep1(j)
            # ---------- boundary exchange ----------
            bcol = bcolp.tile([P, 1], f32, tag="bcol")
            nc.vector.tensor_copy(bcol[:, :], env_last[:, TF - 1:TF])
            st1 = nc.sync.dma_start(bnd_loc[0:64], bcol[64:128, 0:1])
            st2 = nc.sync.dma_start(bnd_loc[64:128], bcol[0:64, 0:1])
            cc = nc.gpsimd.collective_compute(
                "AllGather", mybir.AluOpType.bypass,
                replica_groups=groups,
                ins=[bnd_loc[:]], outs=[bnd_all[:, :]],
            )
            add_dep_helper(cc.ins, st1.ins, sync=True,
                           reason="collective after bnd stores")
            add_dep_helper(cc.ins, st2.ins, sync=True,
                           reason="collective after bnd stores")
            for g in range(NCORES):
                ld = nc.sync.dma_start(bnd_sb[:, g:g + 1], bnd_all[g, :])
                add_dep_helper(ld.ins, cc.ins, sync=True,
                               reason="bnd load after collective")
            nc.vector.tensor_tensor(sel_t[:, :], bnd_sb[:, :], selw_sb[:, :],
                                    op=Alu.mult)
            seed_t = seedp.tile([P, 1], f32, tag="sx")
            nc.vector.tensor_reduce(seed_t[:, :], sel_t[:, :],
                                    axis=mybir.AxisListType.X, op=Alu.add)

            # ---------- solve 1 (final) ----------
            seedy_t = seedp.tile([P, 1], f32, tag="sy")
            nc.vector.tensor_scalar(seedy_t[:, :], seed_t[:, :], float(YSf),
                                    None, op0=Alu.mult)
            prev_y = None
            for j in range(NT):
                a = j * TF
                y_t = yhp.tile([P, TF], f16, tag="yh")
                init_ap = (prev_y[:, TF - 1:TF] if j > 0 else seedy_t[:, 0:1])
                nc.vector.tensor_tensor_scan(y_t[:, :], prep1[j]["c"][:, :],
                                             prep1[j]["d1"][:, :], init_ap,
                                             op0=Alu.mult, op1=Alu.add)
                nc.gpsimd.dma_start(y_out[:, a:a + TF], y_t[:, :])
                prev_y = y_t
                prep1.pop(j, None)
                if j + LOOK < NT:
                    emit_prep1(j + LOOK)
    nc.finalize()
    return nc


def _in_maps(x, ca, cr):
    x = np.ascontiguousarray(np.asarray(x, dtype=np.float32))
    maps = []
    t0 = np.empty(P, np.float64)
    for c in range(NCORES):
        t0[:64] = c * KCORE
        t0[64:] = c * KCORE + HALF
        seed0 = (EQ * (1.0 - np.exp(-t0 / TAU))).astype(np.float32)[:, None]
        selw = np.zeros((P, NCORES), np.float32)
        if c > 0:
            selw[:64, c - 1] = 1.0
        selw[64:, c] = 1.0
        s = c * KCORE
        xc = np.concatenate([x[:, s:s + HALF], x[:, s + HALF:s + KCORE]],
                            axis=0)
        maps.append({
            "xc": np.ascontiguousarray(xc),
            "seed0": seed0,
            "selw": selw,
        })
    return maps


def kernel(x, raw_attack, raw_release, sample_rate):
    from concourse.bass_utils import run_bass_kernel_spmd

    ca, cr = _coeffs(raw_attack, raw_release, sample_rate)
    key = (round(ca, 12), round(cr, 12), TF, ITERS, GAMMA, CM_SAMPLES)
    if key not in _cache:
        _cache[key] = _build(ca, cr)
    nc = _cache[key]

    maps = _in_maps(x, ca, cr)
    res = run_bass_kernel_spmd(nc, maps, list(range(NCORES)), **_RUN_KWARGS)
    kernel.last_results = res

    y = np.empty((B, L), np.float32)
    for c in range(NCORES):
        yc = np.asarray(res.results[c]["yc"], dtype=np.float32) * np.float32(1.0 / YS)
        s = c * KCORE
        y[:, s:s + HALF] = yc[:64]
        y[:, s + HALF:s + KCORE] = yc[64:]
    return y
